# revision 2
# baseline (speedup 1.0000x reference)
"""Channel-wise FC kernel for Trainium2 (8 NeuronCores, SPMD).

Problem: out[b,c] = x[b,c] @ weights[c].T + bias[c]
  x: (8, 32, 1024, 512) f32, weights: (32, 512, 512) f32, bias: (32, 512) f32

Sharding: channel-parallel — core i owns channels [4i, 4i+4). Each core runs
an identical program over its 4 channels; for each channel it computes
YT[f, bn] = W^T[g,f].T-contracted with XT[g, bn] (+bias), i.e. the output is
produced transposed (f-major). Host does the (free) layout transposes.

Compute dtype bf16 (TensorE full rate), accumulation fp32 in PSUM, outputs
stored bf16 and upcast on host.
"""

import os
import sys

for _p in ("/root/.axon_site/_ro/trn_rl_repo", "/opt/trn_rl_repo"):
    if os.path.isdir(_p) and _p not in sys.path:
        sys.path.append(_p)

import numpy as np
import ml_dtypes

B, C, N, F, G = 8, 32, 1024, 512, 512
NCORES = 8
C_LOC = C // NCORES          # 4 channels per core
BN = B * N                   # 8192 rows per channel
P = 128
GT = G // P                  # 4 contraction tiles
FT = F // P                  # 4 output-partition tiles
NCH = 512                    # rows per matmul chunk
N_CHUNKS = BN // NCH         # 16

_BF16 = ml_dtypes.bfloat16

_compiled = None


def _build():
    import concourse.bacc as bacc
    import concourse.mybir as mybir
    import concourse.tile as tile

    BF16 = mybir.dt.bfloat16
    F32 = mybir.dt.float32

    nc = bacc.Bacc("TRN2", target_bir_lowering=False, debug=False)
    xt = nc.dram_tensor("xt", [C_LOC, G, BN], BF16, kind="ExternalInput")
    wt = nc.dram_tensor("wt", [C_LOC, G, F], BF16, kind="ExternalInput")
    bias = nc.dram_tensor("bias", [C_LOC, F], F32, kind="ExternalInput")
    out = nc.dram_tensor("out", [C_LOC, F, BN], BF16, kind="ExternalOutput")

    xt_r = xt.ap().rearrange("c (gt p) bn -> c p gt bn", p=P)
    wt_r = wt.ap().rearrange("c (gt p) f -> c p gt f", p=P)
    bias_r = bias.ap().rearrange("c (ft p) -> p (c ft)", p=P)
    out_ap = out.ap()

    with tile.TileContext(nc) as tc:
        with (
            tc.tile_pool(name="wpool", bufs=2) as wpool,
            tc.tile_pool(name="xpool", bufs=3) as xpool,
            tc.tile_pool(name="opool", bufs=6) as opool,
            tc.tile_pool(name="bpool", bufs=1) as bpool,
            tc.tile_pool(name="psum", bufs=8, space="PSUM") as pspool,
        ):
            b_sb = bpool.tile([P, C_LOC * FT], F32)
            nc.sync.dma_start(b_sb[:], bias_r)
            for c in range(C_LOC):
                w_sb = wpool.tile([P, GT, F], BF16)
                nc.sync.dma_start(w_sb[:], wt_r[c])
                for nb in range(N_CHUNKS):
                    x_sb = xpool.tile([P, GT, NCH], BF16)
                    nc.sync.dma_start(
                        x_sb[:], xt_r[c][:, :, nb * NCH:(nb + 1) * NCH]
                    )
                    for ft in range(FT):
                        ps = pspool.tile([P, NCH], F32)
                        for gt in range(GT):
                            nc.tensor.matmul(
                                ps[:],
                                w_sb[:, gt, ft * P:(ft + 1) * P],
                                x_sb[:, gt, :],
                                start=(gt == 0),
                                stop=(gt == GT - 1),
                            )
                        o_sb = opool.tile([P, NCH], BF16)
                        bcol = b_sb[:, c * FT + ft:c * FT + ft + 1]
                        if ft % 2 == 0:
                            nc.scalar.activation(
                                o_sb[:], ps[:],
                                mybir.ActivationFunctionType.Identity,
                                bias=bcol,
                            )
                        else:
                            nc.vector.tensor_scalar_add(o_sb[:], ps[:], bcol)
                        nc.sync.dma_start(
                            out_ap[c][ft * P:(ft + 1) * P, nb * NCH:(nb + 1) * NCH],
                            o_sb[:],
                        )
    nc.compile()
    return nc


def _get_compiled():
    global _compiled
    if _compiled is None:
        _compiled = _build()
    return _compiled


def _shard_inputs(x, weights, bias):
    """Host-side: slice channels per core and lay out transposed bf16 views."""
    in_maps = []
    for i in range(NCORES):
        sl = slice(i * C_LOC, (i + 1) * C_LOC)
        # x[:, sl]: (B, C_LOC, N, G) -> (C_LOC, G, B, N) -> (C_LOC, G, BN)
        xt = np.ascontiguousarray(
            x[:, sl].transpose(1, 3, 0, 2)
        ).reshape(C_LOC, G, BN).astype(_BF16)
        wt = np.ascontiguousarray(
            weights[sl].transpose(0, 2, 1)
        ).astype(_BF16)
        bs = np.ascontiguousarray(bias[sl], dtype=np.float32)
        in_maps.append({"xt": xt, "wt": wt, "bias": bs})
    return in_maps


def _unshard_output(results):
    # per-core out: (C_LOC, F, BN) bf16, f-major transposed
    yt = np.stack([np.asarray(r["out"]) for r in results])   # (8, C_LOC, F, BN)
    yt = yt.reshape(C, F, B, N)                              # (C, F, B, N)
    y = yt.transpose(2, 0, 3, 1).astype(np.float32)          # (B, C, N, F)
    return np.ascontiguousarray(y)


def run_on_device(in_maps, **kwargs):
    from concourse.bass_utils import run_bass_kernel_spmd

    nc = _get_compiled()
    return run_bass_kernel_spmd(nc, in_maps, core_ids=list(range(NCORES)), **kwargs)


def kernel(x, weights, bias):
    x = np.asarray(x, dtype=np.float32)
    weights = np.asarray(weights, dtype=np.float32)
    bias = np.asarray(bias, dtype=np.float32)
    in_maps = _shard_inputs(x, weights, bias)
    res = run_on_device(in_maps)
    return _unshard_output(res.results)


# revision 6
# speedup vs baseline: 1.2724x; 1.2724x over previous
"""Channel-wise FC kernel for Trainium2 (8 NeuronCores, SPMD).

Problem: out[b,c] = x[b,c] @ weights[c].T + bias[c]
  x: (8, 32, 1024, 512) f32, weights: (32, 512, 512) f32, bias: (32, 512) f32

Sharding: channel-parallel — core i owns channels [4i, 4i+4). Each core runs
an identical program over its 4 channels; for each channel it computes
YT[f, bn] = W^T[g,f].T-contracted with XT[g, bn] (+bias), i.e. the output is
produced transposed (f-major). Host does the (free) layout transposes.

Compute dtype bf16 (TensorE full rate), accumulation fp32 in PSUM, outputs
stored bf16 and upcast on host.
"""

import os
import sys

for _p in ("/root/.axon_site/_ro/trn_rl_repo", "/opt/trn_rl_repo"):
    if os.path.isdir(_p) and _p not in sys.path:
        sys.path.append(_p)

import numpy as np
import ml_dtypes

B, C, N, F, G = 8, 32, 1024, 512, 512
NCORES = 8
C_LOC = C // NCORES          # 4 channels per core
BN = B * N                   # 8192 rows per channel
P = 128
GT = G // P                  # 4 contraction tiles
FT = F // P                  # 4 output-partition tiles
NCH = 2048                   # rows per x DMA chunk (4KB partition lines)
N_CHUNKS = BN // NCH         # 4
NSL = NCH // 512             # 512-row matmul slices per chunk
OCH = 2 * NCH                # rows per out store tile (1MB transfers)

_BF16 = ml_dtypes.bfloat16

_compiled = None


def _build():
    import concourse.bacc as bacc
    import concourse.mybir as mybir
    import concourse.tile as tile

    BF16 = mybir.dt.bfloat16
    F32 = mybir.dt.float32

    nc = bacc.Bacc("TRN2", target_bir_lowering=False, debug=False)
    xt = nc.dram_tensor("xt", [C_LOC, G, BN], BF16, kind="ExternalInput")
    wt = nc.dram_tensor("wt", [C_LOC, G, F], BF16, kind="ExternalInput")
    bias = nc.dram_tensor("bias", [C_LOC, F], F32, kind="ExternalInput")
    out = nc.dram_tensor("out", [C_LOC, F, BN], BF16, kind="ExternalOutput")

    xt_r = xt.ap().rearrange("c (gt p) bn -> c p gt bn", p=P)
    wt_r = wt.ap().rearrange("c (gt p) f -> c p gt f", p=P)
    bias_r = bias.ap().rearrange("c (ft p) -> p (c ft)", p=P)
    out_ap = out.ap()

    with tile.TileContext(nc) as tc:
        with (
            tc.tile_pool(name="wpool", bufs=2) as wpool,
            tc.tile_pool(name="xpool", bufs=3) as xpool,
            tc.tile_pool(name="opool", bufs=2) as opool,
            tc.tile_pool(name="bpool", bufs=1) as bpool,
            tc.tile_pool(name="psum", bufs=8, space="PSUM") as pspool,
        ):
            b_sb = bpool.tile([P, C_LOC * FT], F32)
            nc.sync.dma_start(b_sb[:], bias_r)
            for c in range(C_LOC):
                w_sb = wpool.tile([P, GT, F], BF16)
                nc.sync.dma_start(w_sb[:], wt_r[c])
                o_sbs = None
                for nb in range(N_CHUNKS):
                    x_sb = xpool.tile([P, GT, NCH], BF16)
                    nc.sync.dma_start(
                        x_sb[:], xt_r[c][:, :, nb * NCH:(nb + 1) * NCH]
                    )
                    if nb % 2 == 0:
                        o_sbs = [
                            opool.tile([P, OCH], BF16, tag=f"o{ft}",
                                       name=f"o{ft}_{c}_{nb}")
                            for ft in range(FT)
                        ]
                    for ns in range(NSL):
                        for ft in range(FT):
                            ps = pspool.tile([P, 512], F32)
                            for gt in range(GT):
                                nc.tensor.matmul(
                                    ps[:],
                                    w_sb[:, gt, ft * P:(ft + 1) * P],
                                    x_sb[:, gt, ns * 512:(ns + 1) * 512],
                                    start=(gt == 0),
                                    stop=(gt == GT - 1),
                                )
                            off = (nb % 2) * NCH + ns * 512
                            oslice = o_sbs[ft][:, off:off + 512]
                            bcol = b_sb[:, c * FT + ft:c * FT + ft + 1]
                            if ft % 2 == 0:
                                nc.scalar.activation(
                                    oslice, ps[:],
                                    mybir.ActivationFunctionType.Identity,
                                    bias=bcol,
                                )
                            else:
                                nc.vector.tensor_scalar_add(oslice, ps[:], bcol)
                    if nb % 2 == 1:
                        n0 = (nb - 1) * NCH
                        for ft in range(FT):
                            nc.sync.dma_start(
                                out_ap[c][ft * P:(ft + 1) * P, n0:n0 + OCH],
                                o_sbs[ft][:],
                            )
    nc.compile()
    return nc


def _get_compiled():
    global _compiled
    if _compiled is None:
        _compiled = _build()
    return _compiled


def _shard_inputs(x, weights, bias):
    """Host-side: slice channels per core and lay out transposed bf16 views."""
    in_maps = []
    for i in range(NCORES):
        sl = slice(i * C_LOC, (i + 1) * C_LOC)
        # x[:, sl]: (B, C_LOC, N, G) -> (C_LOC, G, B, N) -> (C_LOC, G, BN)
        xt = np.ascontiguousarray(
            x[:, sl].transpose(1, 3, 0, 2)
        ).reshape(C_LOC, G, BN).astype(_BF16)
        wt = np.ascontiguousarray(
            weights[sl].transpose(0, 2, 1)
        ).astype(_BF16)
        bs = np.ascontiguousarray(bias[sl], dtype=np.float32)
        in_maps.append({"xt": xt, "wt": wt, "bias": bs})
    return in_maps


def _unshard_output(results):
    # per-core out: (C_LOC, F, BN) bf16, f-major transposed
    yt = np.stack([np.asarray(r["out"]) for r in results])   # (8, C_LOC, F, BN)
    yt = yt.reshape(C, F, B, N)                              # (C, F, B, N)
    y = yt.transpose(2, 0, 3, 1).astype(np.float32)          # (B, C, N, F)
    return np.ascontiguousarray(y)


def run_on_device(in_maps, **kwargs):
    from concourse.bass_utils import run_bass_kernel_spmd

    nc = _get_compiled()
    return run_bass_kernel_spmd(nc, in_maps, core_ids=list(range(NCORES)), **kwargs)


def kernel(x, weights, bias):
    x = np.asarray(x, dtype=np.float32)
    weights = np.asarray(weights, dtype=np.float32)
    bias = np.asarray(bias, dtype=np.float32)
    in_maps = _shard_inputs(x, weights, bias)
    res = run_on_device(in_maps)
    return _unshard_output(res.results)
